# revision 8
# baseline (speedup 1.0000x reference)
"""Cached GQA attention block (B=4, Q=512, D=4096, H=32, KVH=8, KV=4096) on 8
Trainium2 NeuronCores.

Sharding: tensor-parallel over heads. Core i owns kv-head i and q-heads
4i..4i+3: it projects Q/K/V for those heads (f32r matmuls at full PE rate),
runs attention against its kv-cache shard, all-gathers the per-head attention
outputs (bf16), and computes output columns 512i..512(i+1) of the W_o
projection. Host only reshapes/shards inputs and concatenates outputs.

All activations use float32r (TF32-like, full PE rate, ~1.5e-4 rel err)
except the all-gather + W_o operands which are bf16 to halve HBM/collective
traffic on the critical tail.
"""
import numpy as np

B, Q, D = 4, 512, 4096
H, KVH, HD = 32, 8, 128
KV = 4096
S = KV + Q              # 4608
NCORES = 8
HLOC = H // NCORES      # 4 q heads per core
TOK = B * Q             # 2048
KC = D // 128           # 32 contraction chunks
SC = S // 128           # 36 key chunks
NPAIR = SC // 2         # 18 key chunk pairs
TC = 8                  # projection t-chunks
TCW = TOK // TC         # 256 tokens per projection chunk
SCALE = HD ** -0.5

_BUILT = {}


def _phase_proj(g):
    nc, tc_ = g["nc"], g["tc"]
    f32, f32r = g["f32"], g["f32r"]
    with (
        nc.named_scope("proj"),
        tc_.tile_pool(name="pw", bufs=1) as pw,
        tc_.tile_pool(name="px", bufs=3) as px,
        tc_.tile_pool(name="pst", bufs=4) as pst,
        tc_.tile_pool(name="pps", bufs=1, space="PSUM") as pps,
        tc_.tile_pool(name="tps", bufs=2, space="PSUM") as tps,
    ):
        wq_all = pw.tile([128, KC * 512], f32r, name="wq_all")
        wk_all = pw.tile([128, KC * HD], f32r, name="wk_all")
        wv_all = pw.tile([128, KC * HD], f32r, name="wv_all")

        def load_wq_quarter(qi):
            nc.gpsimd.dma_start(
                wq_all[:].rearrange("p (k m) -> p k m", k=KC)[:, qi * 8:(qi + 1) * 8],
                g["wq_d"].rearrange("(k p) m -> p k m", p=128)[:, qi * 8:(qi + 1) * 8],
            )

        def load_wkv():
            nc.gpsimd.dma_start(
                wk_all[:].rearrange("p (k m) -> p k m", k=KC),
                g["wk_d"].rearrange("(k p) m -> p k m", p=128),
            )
            nc.gpsimd.dma_start(
                wv_all[:].rearrange("p (k m) -> p k m", k=KC),
                g["wv_d"].rearrange("(k p) m -> p k m", p=128),
            )

        for t in range(TC):
            xq = []
            for qi in range(2):
                xt_ = px.tile([128, 16 * TCW], f32r, name=f"x{t}_{qi}", tag="xh")
                nc.gpsimd.dma_start(
                    xt_[:].rearrange("p (k t) -> p k t", k=16),
                    g["xT_d"].rearrange("(k p) t -> p k t", p=128)[
                        :, qi * 16:(qi + 1) * 16, t * TCW:(t + 1) * TCW],
                )
                xq.append(xt_)
                if t == 0:
                    load_wq_quarter(2 * qi)
                    load_wq_quarter(2 * qi + 1)
            if t == 0:
                load_wkv()

            def xk(k):
                return xq[k // 16][:, (k % 16) * TCW:(k % 16 + 1) * TCW]

            for h in range(HLOC):
                ph = pps.tile([128, TCW], f32, name=f"pq{t}_{h}", tag=f"pq{h}")
                for k in range(KC):
                    nc.tensor.matmul(
                        ph[:], wq_all[:, k * 512 + h * HD:k * 512 + (h + 1) * HD],
                        xk(k), start=(k == 0), stop=(k == KC - 1))
                st = pst.tile([128, TCW], f32r, name=f"sq{t}_{h}", tag="sq")
                nc.vector.tensor_copy(st[:], ph[:])
                nc.sync.dma_start(g["qt_d"][h, :, t * TCW:(t + 1) * TCW], st[:])

            pk = pps.tile([128, TCW], f32, name=f"pk{t}", tag="pk")
            pv = pps.tile([128, TCW], f32, name=f"pv{t}", tag="pv")
            for k in range(KC):
                nc.tensor.matmul(pk[:], wk_all[:, k * HD:(k + 1) * HD],
                                 xk(k), start=(k == 0), stop=(k == KC - 1))
            for k in range(KC):
                nc.tensor.matmul(pv[:], wv_all[:, k * HD:(k + 1) * HD],
                                 xk(k), start=(k == 0), stop=(k == KC - 1))
            nc.vector.tensor_copy(g["kt_new"][:, t * TCW:(t + 1) * TCW], pk[:])
            nc.vector.tensor_copy(g["vt_new"][:, t * TCW:(t + 1) * TCW], pv[:])

        # transposes: v natural (attention lhsT + vnew out), k natural (knew out)
        for j in range(TOK // 128):
            sl = slice(j * 128, (j + 1) * 128)
            tv = tps.tile([128, 128], f32, name=f"tv{j}", tag="tp")
            nc.tensor.transpose(tv[:], g["vt_new"][:, sl].bitcast(f32), g["ident"][:])
            nc.vector.tensor_copy(g["v_nat"][:, sl], tv[:])
            nc.sync.dma_start(g["vnew_d"][sl, :], g["v_nat"][:, sl].bitcast(f32))
            tk = tps.tile([128, 128], f32, name=f"tk{j}", tag="tp")
            nc.tensor.transpose(tk[:], g["kt_new"][:, sl].bitcast(f32), g["ident"][:])
            ks = pst.tile([128, 128], f32, name=f"ks{j}", tag="ks")
            nc.vector.tensor_copy(ks[:], tk[:])
            nc.sync.dma_start(g["knew_d"][sl, :], ks[:])


def _attn_head(g, pools, b, h, ktf, vf, pools_qt):
    nc = g["nc"]
    mybir = g["mybir"]
    f32, f32r, bf16 = g["f32"], g["f32r"], g["bf16"]
    EXP = mybir.ActivationFunctionType.Exp
    qp, ep, tp2, rp, onp, scps, ops, dps = pools
    NKV = KV // 128

    def kslice(c):
        if c < NKV:
            return ktf[:, c * 128:(c + 1) * 128]
        j = c - NKV
        return g["kt_new"][:, b * Q + j * 128:b * Q + (j + 1) * 128]

    def vslice(c):
        if c < NKV:
            return vf[:, c * 128:(c + 1) * 128]
        j = c - NKV
        return g["v_nat"][:, (b * 4 + j) * 128:(b * 4 + j + 1) * 128]

    qt = pools_qt[h]
    out_acc = ops.tile([128, Q], f32, name=f"oa{b}_{h}", tag="oa")
    den_acc = dps.tile([128, Q], f32, name=f"da{b}_{h}", tag="da")
    for pr in range(NPAIR):
        sc = scps.tile([128, 1024], f32, name=f"sc{b}_{h}_{pr}", tag="sc")
        for hf in range(2):
            c = 2 * pr + hf
            nc.tensor.matmul(sc[:, hf * Q:(hf + 1) * Q], kslice(c), qt[:],
                             start=True, stop=True)
        et = ep.tile([128, 1024], f32r, name=f"et{b}_{h}_{pr}", tag="et")
        if pr < NPAIR - 2:
            nc.scalar.activation(et[:], sc[:], EXP, scale=SCALE)
        else:
            etm = tp2.tile([128, 1024], f32, name=f"etm{b}_{h}_{pr}", tag="etm")
            nc.scalar.activation(etm[:], sc[:], EXP, scale=SCALE)
            mi = pr - (NPAIR - 2)
            nc.vector.tensor_mul(et[:], etm[:],
                                 g["mask_t"][:, mi * 1024:(mi + 1) * 1024])
        for hf in range(2):
            c = 2 * pr + hf
            nc.tensor.matmul(out_acc[:], vslice(c), et[:, hf * Q:(hf + 1) * Q],
                             start=(c == 0), stop=(c == SC - 1))
            last_mm = nc.tensor.matmul(
                den_acc[:], g["ones_c"][:], et[:, hf * Q:(hf + 1) * Q],
                start=(c == 0), stop=(c == SC - 1))
    dsb = rp.tile([128, Q], f32, name=f"ds{b}_{h}", tag="ds")
    nc.vector.tensor_copy(dsb[:], den_acc[:])
    oac = rp.tile([128, Q], f32, name=f"oc{b}_{h}", tag="oc")
    nc.vector.tensor_copy(oac[:], out_acc[:])
    rb = rp.tile([128, Q], f32, name=f"rb{b}_{h}", tag="rb")
    nc.vector.reciprocal(rb[:], dsb[:])
    outn = onp.tile([128, Q], bf16, name=f"on{b}_{h}", tag="on")
    nc.vector.tensor_mul(outn[:], oac[:], rb[:])
    nc.sync.dma_start(g["ag_in"][b][h * 128:(h + 1) * 128, :], outn[:])
    return last_mm


def _phase_attn(g, pools, ka, va):
    nc, mybir = g["nc"], g["mybir"]
    f32r = g["f32r"]
    qp = pools[0]

    def load_kv(b):
        ktf = ka.tile([128, KV], f32r, name=f"ktf{b}", tag="ktf")
        nc.gpsimd.dma_start(ktf[:], g["ktc_d"][b])
        vf = va.tile([128, KV], f32r, name=f"vf{b}", tag="vf")
        nc.gpsimd.dma_start(
            vf[:].rearrange("p (c h) -> p c h", c=KV // 128),
            g["vc_d"][b].rearrange("(c p) h -> p c h", p=128),
        )
        return ktf, vf

    def load_qt(b):
        qta = qp.tile([128, HLOC * Q], f32r, name=f"qta{b}", tag="qt")
        ld = nc.sync.dma_start(
            qta[:].rearrange("p (h q) -> p h q", h=HLOC),
            g["qt_d"][:, :, b * Q:(b + 1) * Q].transpose([1, 0, 2]),
        )
        return [qta[:, h * Q:(h + 1) * Q] for h in range(HLOC)], ld

    with nc.named_scope("attn"):
        kv = load_kv(0)
        qts, _ = load_qt(0)
        prev_wo_last = None
        for b in range(B):
            # issue b+1's loads BEFORE anything in batch b can block the queues
            kv_next = load_kv(b + 1) if b + 1 < B else None
            qt_ld = None
            if b + 1 < B:
                qts_next, qt_ld = load_qt(b + 1)
            else:
                qts_next = None
            last_mm = None
            ktf, vf = kv
            for h in range(HLOC):
                last_mm = _attn_head(g, pools, b, h, ktf, vf, qts)
            kv, qts = kv_next, qts_next
            nc.gpsimd.collective_compute(
                "AllGather",
                mybir.AluOpType.bypass,
                replica_groups=[list(range(NCORES))],
                ins=[g["ag_in"][b].opt()],
                outs=[g["ag_out"][b].opt()],
            )
            # W_o for batch b-1: keep it AFTER batch b's attention on the PE
            # queue (the scheduler otherwise hoists it and stalls on the AG)
            if b >= 1:
                prev_wo_last = g["wo_batch"](b - 1, last_mm, qt_ld, prev_wo_last)
        g["wo_batch"](3, None, None, prev_wo_last)


def _make_wo_batch(g, wo_all, agp, osp, wps):
    nc = g["nc"]
    from concourse.tile import add_dep_helper
    f32, bf16 = g["f32"], g["bf16"]

    def wo_batch(b, after_mm, after_dma, prev_last):
        last = None
        with nc.named_scope("wo"):
            for pass_ in range(2):
                po = [wps.tile([128, 512], f32, name=f"po{b}_{pass_}_{m}",
                               tag=f"po{m}") for m in range(2)]
                # 8 batched agt loads (4 k-chunks each) instead of 32
                ags = []
                for kg in range(8):
                    agt = agp.tile([128, 4 * 256], bf16,
                                   name=f"agt{b}_{pass_}_{kg}", tag="agt")
                    ld = nc.sync.dma_start(
                        agt[:].rearrange("p (k q) -> p k q", k=4),
                        g["ag_out"][b].rearrange("(k p) q -> p k q", p=128)[
                            :, kg * 4:(kg + 1) * 4,
                            pass_ * 256:(pass_ + 1) * 256],
                    )
                    if kg == 0 and pass_ == 0 and after_dma is not None:
                        add_dep_helper(ld.ins, after_dma.ins, sync=False,
                                       reason="agt loads after next-batch qt")
                    ags.append(agt)
                for k in range(KC):
                    agt = ags[k // 4]
                    coff = (k % 4) * 256
                    for m in range(2):
                        mm = nc.tensor.matmul(
                            po[m][:], agt[:, coff + m * 128:coff + (m + 1) * 128],
                            wo_all[:, k * 512:(k + 1) * 512],
                            start=(k == 0), stop=(k == KC - 1))
                        if k == 0:
                            for dep in (after_mm, prev_last):
                                if dep is not None:
                                    add_dep_helper(mm.ins, dep.ins, sync=False,
                                                   reason="wo after attention")
                        last = mm
                for m in range(2):
                    ost = osp.tile([128, 512], f32,
                                   name=f"os{b}_{pass_}_{m}", tag="os")
                    nc.vector.tensor_copy(ost[:], po[m][:])
                    row = (b * 4 + pass_ * 2 + m) * 128
                    nc.sync.dma_start(g["outp_d"][row:row + 128, :], ost[:])
        return last

    return wo_batch


def _body(g):
    nc, tc_ = g["nc"], g["tc"]
    f32, f32r, bf16 = g["f32"], g["f32r"], g["bf16"]
    pp, dp = g["pp"], g["dp"]

    g["kt_new"] = pp.tile([128, TOK], f32r, name="kt_new")   # (hd, t)
    g["vt_new"] = pp.tile([128, TOK], f32r, name="vt_new")   # (hd, t)
    g["v_nat"] = pp.tile([128, TOK], f32r, name="v_nat")     # (t%128, [tc, hd])
    mask_t = pp.tile([128, 2 * 1024], f32r, name="mask_t")
    nc.gpsimd.dma_start(
        mask_t[:].rearrange("p (c q) -> p c q", c=2),
        g["mk_d"].rearrange("c p q -> p c q"),
    )
    g["mask_t"] = mask_t
    ones_f = pp.tile([128, 128], f32, name="ones_f")
    nc.vector.memset(ones_f[:], 1.0)
    ones_c = pp.tile([128, 128], f32r, name="ones_c")
    nc.vector.tensor_copy(ones_c[:], ones_f[:])
    g["ones_c"] = ones_c
    ident = pp.tile([128, 128], f32, name="ident")
    g["make_identity"](nc, ident)
    g["ident"] = ident

    g["qt_d"] = dp.tile([HLOC, 128, TOK], f32r, name="qt_d")
    g["ag_in"] = [dp.tile([512, 512], bf16, name=f"ag_in{b}") for b in range(B)]
    g["ag_out"] = [
        dp.tile([D, 512], bf16, name=f"ag_out{b}", addr_space="Shared")
        for b in range(B)
    ]

    _phase_proj(g)

    with (
        tc_.tile_pool(name="ka", bufs=2) as ka,
        tc_.tile_pool(name="va", bufs=2) as va,
        tc_.tile_pool(name="ep", bufs=4) as ep,
        tc_.tile_pool(name="tp2", bufs=2) as tp2,
        tc_.tile_pool(name="qp", bufs=3) as qp,
        tc_.tile_pool(name="rp", bufs=2) as rp,
        tc_.tile_pool(name="onp", bufs=2) as onp,
        tc_.tile_pool(name="wop", bufs=1) as wop,
        tc_.tile_pool(name="agp", bufs=6) as agp,
        tc_.tile_pool(name="osp", bufs=2) as osp,
    ):
        wo_all = wop.tile([128, KC * 512], bf16, name="wo_all")
        nc.gpsimd.dma_start(
            wo_all[:].rearrange("p (k m) -> p k m", k=KC),
            g["wo_d"].rearrange("(k p) m -> p k m", p=128),
        )
        with (
            tc_.tile_pool(name="scps", bufs=2, space="PSUM") as scps,
            tc_.tile_pool(name="ops", bufs=1, space="PSUM") as ops,
            tc_.tile_pool(name="dps", bufs=1, space="PSUM") as dps,
            tc_.tile_pool(name="wps", bufs=1, space="PSUM") as wps,
        ):
            g["wo_batch"] = _make_wo_batch(g, wo_all, agp, osp, wps)
            pools = (qp, ep, tp2, rp, onp, scps, ops, dps)
            _phase_attn(g, pools, ka, va)


def _build():
    import concourse.mybir as mybir
    import concourse.tile as tile
    from concourse import bacc
    from concourse.masks import make_identity

    f32 = mybir.dt.float32
    f32r = mybir.dt.float32r
    bf16 = mybir.dt.bfloat16

    nc = bacc.Bacc("TRN2", target_bir_lowering=False, debug=False,
                   num_devices=NCORES)

    g = {
        "nc": nc, "mybir": mybir, "make_identity": make_identity,
        "f32": f32, "f32r": f32r, "bf16": bf16,
        "xT_d": nc.dram_tensor("xT", [D, TOK], f32, kind="ExternalInput").ap(),
        "wq_d": nc.dram_tensor("wq", [D, HLOC * HD], f32, kind="ExternalInput").ap(),
        "wk_d": nc.dram_tensor("wk", [D, HD], f32, kind="ExternalInput").ap(),
        "wv_d": nc.dram_tensor("wv", [D, HD], f32, kind="ExternalInput").ap(),
        "ktc_d": nc.dram_tensor("ktc", [B, HD, KV], f32, kind="ExternalInput").ap(),
        "vc_d": nc.dram_tensor("vc", [B, KV, HD], f32, kind="ExternalInput").ap(),
        "wo_d": nc.dram_tensor("wo", [D, 512], f32, kind="ExternalInput").ap(),
        "mk_d": nc.dram_tensor("mk", [2, 128, 1024], f32, kind="ExternalInput").ap(),
        "outp_d": nc.dram_tensor("outp", [TOK, 512], f32, kind="ExternalOutput").ap(),
        "knew_d": nc.dram_tensor("knew", [TOK, HD], f32, kind="ExternalOutput").ap(),
        "vnew_d": nc.dram_tensor("vnew", [TOK, HD], f32, kind="ExternalOutput").ap(),
    }

    with tile.TileContext(nc) as tc_:
        g["tc"] = tc_
        with (
            tc_.tile_pool(name="persist", bufs=1) as pp,
            tc_.tile_pool(name="dram", bufs=1, space="DRAM") as dp,
        ):
            g["pp"], g["dp"] = pp, dp
            _body(g)

    nc.compile()
    return nc


def _get_nc():
    if "nc" not in _BUILT:
        _BUILT["nc"] = _build()
    return _BUILT["nc"]


def _host_prep(x, k_cache, v_cache, W_q, W_k, W_v, W_o):
    x = np.asarray(x, np.float32)
    k_cache = np.asarray(k_cache, np.float32)
    v_cache = np.asarray(v_cache, np.float32)
    W_q = np.asarray(W_q, np.float32)
    W_k = np.asarray(W_k, np.float32)
    W_v = np.asarray(W_v, np.float32)
    W_o = np.asarray(W_o, np.float32)

    xT = np.ascontiguousarray(x.reshape(TOK, D).T)
    ktc = np.ascontiguousarray(k_cache.transpose(0, 1, 3, 2))  # (B, KVH, HD, KV)

    # causal mask pair-tiles for the last 4 key chunks
    q_idx = np.arange(Q)
    mk = np.zeros((2, 128, 1024), np.float32)
    for p_ in range(2):
        for hf in range(2):
            c_local = p_ * 2 + hf
            j_glob = c_local * 128 + np.arange(128)
            mk[p_, :, hf * Q:(hf + 1) * Q] = (
                q_idx[None, :] >= j_glob[:, None]).astype(np.float32)

    in_maps = []
    for i in range(NCORES):
        in_maps.append({
            "xT": xT,
            "wq": np.ascontiguousarray(W_q[:, i * 512:(i + 1) * 512]),
            "wk": np.ascontiguousarray(W_k[:, i * HD:(i + 1) * HD]),
            "wv": np.ascontiguousarray(W_v[:, i * HD:(i + 1) * HD]),
            "ktc": np.ascontiguousarray(ktc[:, i]),
            "vc": np.ascontiguousarray(v_cache[:, i]),
            "wo": np.ascontiguousarray(W_o[:, i * 512:(i + 1) * 512]),
            "mk": mk,
        })
    return in_maps, k_cache, v_cache


def _assemble(results, k_cache, v_cache):
    out = np.empty((TOK, D), np.float32)
    k_new = np.empty((B, KVH, Q, HD), np.float32)
    v_new = np.empty((B, KVH, Q, HD), np.float32)
    for i, r in enumerate(results):
        out[:, i * 512:(i + 1) * 512] = r["outp"]
        k_new[:, i] = r["knew"].reshape(B, Q, HD)
        v_new[:, i] = r["vnew"].reshape(B, Q, HD)
    k_full = np.concatenate([k_cache, k_new], axis=2)
    v_full = np.concatenate([v_cache, v_new], axis=2)
    return out.reshape(B, Q, D), k_full, v_full


def run_spmd(in_maps, **kwargs):
    from concourse import bass_utils
    nc = _get_nc()
    return bass_utils.run_bass_kernel_spmd(
        nc, in_maps, core_ids=list(range(NCORES)), **kwargs)


def kernel(x, k_cache, v_cache, W_q, W_k, W_v, W_o):
    in_maps, kc, vc = _host_prep(x, k_cache, v_cache, W_q, W_k, W_v, W_o)
    res = run_spmd(in_maps)
    return _assemble(res.results, kc, vc)


# revision 9
# speedup vs baseline: 1.1380x; 1.1380x over previous
"""Cached GQA attention block (B=4, Q=512, D=4096, H=32, KVH=8, KV=4096) on 8
Trainium2 NeuronCores.

Sharding: tensor-parallel over heads. Core i owns kv-head i and q-heads
4i..4i+3: it projects Q/K/V for those heads (f32r matmuls at full PE rate),
runs attention against its kv-cache shard, all-gathers the per-head attention
outputs (bf16), and computes output columns 512i..512(i+1) of the W_o
projection. Host only reshapes/shards inputs and concatenates outputs.

All activations use float32r (TF32-like, full PE rate, ~1.5e-4 rel err)
except the all-gather + W_o operands which are bf16 to halve HBM/collective
traffic on the critical tail.
"""
import numpy as np

B, Q, D = 4, 512, 4096
H, KVH, HD = 32, 8, 128
KV = 4096
S = KV + Q              # 4608
NCORES = 8
HLOC = H // NCORES      # 4 q heads per core
TOK = B * Q             # 2048
KC = D // 128           # 32 contraction chunks
SC = S // 128           # 36 key chunks
NPAIR = SC // 2         # 18 key chunk pairs
TC = 8                  # projection t-chunks
TCW = TOK // TC         # 256 tokens per projection chunk
SCALE = HD ** -0.5

_BUILT = {}


def _phase_proj(g):
    nc, tc_ = g["nc"], g["tc"]
    f32, f32r = g["f32"], g["f32r"]
    with (
        nc.named_scope("proj"),
        tc_.tile_pool(name="pw", bufs=1) as pw,
        tc_.tile_pool(name="px", bufs=3) as px,
        tc_.tile_pool(name="pst", bufs=4) as pst,
        tc_.tile_pool(name="pps", bufs=1, space="PSUM") as pps,
        tc_.tile_pool(name="tps", bufs=2, space="PSUM") as tps,
    ):
        wq_all = pw.tile([128, KC * 512], f32r, name="wq_all")
        wk_all = pw.tile([128, KC * HD], f32r, name="wk_all")
        wv_all = pw.tile([128, KC * HD], f32r, name="wv_all")

        def load_wq_quarter(qi):
            nc.gpsimd.dma_start(
                wq_all[:].rearrange("p (k m) -> p k m", k=KC)[:, qi * 8:(qi + 1) * 8],
                g["wq_d"].rearrange("(k p) m -> p k m", p=128)[:, qi * 8:(qi + 1) * 8],
            )

        def load_wkv():
            nc.gpsimd.dma_start(
                wk_all[:].rearrange("p (k m) -> p k m", k=KC),
                g["wk_d"].rearrange("(k p) m -> p k m", p=128),
            )
            nc.gpsimd.dma_start(
                wv_all[:].rearrange("p (k m) -> p k m", k=KC),
                g["wv_d"].rearrange("(k p) m -> p k m", p=128),
            )

        for t in range(TC):
            xq = []
            for qi in range(2):
                xt_ = px.tile([128, 16 * TCW], f32r, name=f"x{t}_{qi}", tag="xh")
                nc.gpsimd.dma_start(
                    xt_[:].rearrange("p (k t) -> p k t", k=16),
                    g["xT_d"].rearrange("(k p) t -> p k t", p=128)[
                        :, qi * 16:(qi + 1) * 16, t * TCW:(t + 1) * TCW],
                )
                xq.append(xt_)
                if t == 0:
                    load_wq_quarter(2 * qi)
                    load_wq_quarter(2 * qi + 1)
            if t == 0:
                load_wkv()

            def xk(k):
                return xq[k // 16][:, (k % 16) * TCW:(k % 16 + 1) * TCW]

            for h in range(HLOC):
                ph = pps.tile([128, TCW], f32, name=f"pq{t}_{h}", tag=f"pq{h}")
                for k in range(KC):
                    nc.tensor.matmul(
                        ph[:], wq_all[:, k * 512 + h * HD:k * 512 + (h + 1) * HD],
                        xk(k), start=(k == 0), stop=(k == KC - 1))
                st = pst.tile([128, TCW], f32r, name=f"sq{t}_{h}", tag="sq")
                nc.vector.tensor_copy(st[:], ph[:])
                nc.sync.dma_start(g["qt_d"][h, :, t * TCW:(t + 1) * TCW], st[:])

            pk = pps.tile([128, TCW], f32, name=f"pk{t}", tag="pk")
            pv = pps.tile([128, TCW], f32, name=f"pv{t}", tag="pv")
            for k in range(KC):
                nc.tensor.matmul(pk[:], wk_all[:, k * HD:(k + 1) * HD],
                                 xk(k), start=(k == 0), stop=(k == KC - 1))
            for k in range(KC):
                nc.tensor.matmul(pv[:], wv_all[:, k * HD:(k + 1) * HD],
                                 xk(k), start=(k == 0), stop=(k == KC - 1))
            nc.vector.tensor_copy(g["kt_new"][:, t * TCW:(t + 1) * TCW], pk[:])
            nc.vector.tensor_copy(g["vt_new"][:, t * TCW:(t + 1) * TCW], pv[:])

        # transposes: v natural (attention lhsT + vnew out), k natural (knew out)
        for j in range(TOK // 128):
            sl = slice(j * 128, (j + 1) * 128)
            tv = tps.tile([128, 128], f32, name=f"tv{j}", tag="tp")
            nc.tensor.transpose(tv[:], g["vt_new"][:, sl].bitcast(f32), g["ident"][:])
            nc.vector.tensor_copy(g["v_nat"][:, sl], tv[:])
            nc.sync.dma_start(g["vnew_d"][sl, :], g["v_nat"][:, sl].bitcast(f32))
            tk = tps.tile([128, 128], f32, name=f"tk{j}", tag="tp")
            nc.tensor.transpose(tk[:], g["kt_new"][:, sl].bitcast(f32), g["ident"][:])
            ks = pst.tile([128, 128], f32, name=f"ks{j}", tag="ks")
            nc.vector.tensor_copy(ks[:], tk[:])
            nc.sync.dma_start(g["knew_d"][sl, :], ks[:])


def _attn_head(g, pools, b, h, ktf, vf, pools_qt):
    nc = g["nc"]
    mybir = g["mybir"]
    f32, f32r, bf16 = g["f32"], g["f32r"], g["bf16"]
    EXP = mybir.ActivationFunctionType.Exp
    qp, ep, tp2, rp, onp, scps, ops, dps = pools
    NKV = KV // 128

    def kslice(c):
        if c < NKV:
            return ktf[:, c * 128:(c + 1) * 128]
        j = c - NKV
        return g["kt_new"][:, b * Q + j * 128:b * Q + (j + 1) * 128]

    def vslice(c):
        if c < NKV:
            return vf[:, c * 128:(c + 1) * 128]
        j = c - NKV
        return g["v_nat"][:, (b * 4 + j) * 128:(b * 4 + j + 1) * 128]

    qt = pools_qt[h]
    out_acc = ops.tile([128, Q], f32, name=f"oa{b}_{h}", tag="oa")
    den_acc = dps.tile([128, Q], f32, name=f"da{b}_{h}", tag="da")
    NKVC = KV // 128
    for c in range(SC):
        sc = scps.tile([128, Q], f32, name=f"sc{b}_{h}_{c}", tag="sc")
        nc.tensor.matmul(sc[:], kslice(c), qt[:], start=True, stop=True)
        et = ep.tile([128, Q], f32r, name=f"et{b}_{h}_{c}", tag="et")
        if c < NKVC:
            nc.scalar.activation(et[:], sc[:], EXP, scale=SCALE)
        else:
            etm = tp2.tile([128, Q], f32, name=f"etm{b}_{h}_{c}", tag="etm")
            nc.scalar.activation(etm[:], sc[:], EXP, scale=SCALE)
            j = c - NKVC
            nc.vector.tensor_mul(et[:], etm[:],
                                 g["mask_t"][:, j * Q:(j + 1) * Q])
        nc.tensor.matmul(out_acc[:], vslice(c), et[:],
                         start=(c == 0), stop=(c == SC - 1))
        last_mm = nc.tensor.matmul(
            den_acc[:], g["ones_c"][:], et[:],
            start=(c == 0), stop=(c == SC - 1))
    dsb = rp.tile([128, Q], f32, name=f"ds{b}_{h}", tag="ds")
    nc.vector.tensor_copy(dsb[:], den_acc[:])
    oac = rp.tile([128, Q], f32, name=f"oc{b}_{h}", tag="oc")
    nc.vector.tensor_copy(oac[:], out_acc[:])
    rb = rp.tile([128, Q], f32, name=f"rb{b}_{h}", tag="rb")
    nc.vector.reciprocal(rb[:], dsb[:])
    outn = onp.tile([128, Q], bf16, name=f"on{b}_{h}", tag="on")
    nc.vector.tensor_mul(outn[:], oac[:], rb[:])
    nc.sync.dma_start(g["ag_in"][b][h * 128:(h + 1) * 128, :], outn[:])
    return last_mm


def _phase_attn(g, pools, ka, va):
    nc, mybir = g["nc"], g["mybir"]
    f32r = g["f32r"]
    qp = pools[0]

    def load_kv(b):
        ktf = ka.tile([128, KV], f32r, name=f"ktf{b}", tag="ktf")
        nc.gpsimd.dma_start(ktf[:], g["ktc_d"][b])
        vf = va.tile([128, KV], f32r, name=f"vf{b}", tag="vf")
        nc.gpsimd.dma_start(
            vf[:].rearrange("p (c h) -> p c h", c=KV // 128),
            g["vc_d"][b].rearrange("(c p) h -> p c h", p=128),
        )
        return ktf, vf

    def load_qt(b):
        qta = qp.tile([128, HLOC * Q], f32r, name=f"qta{b}", tag="qt")
        ld = nc.sync.dma_start(
            qta[:].rearrange("p (h q) -> p h q", h=HLOC),
            g["qt_d"][:, :, b * Q:(b + 1) * Q].transpose([1, 0, 2]),
        )
        return [qta[:, h * Q:(h + 1) * Q] for h in range(HLOC)], ld

    with nc.named_scope("attn"):
        kv = load_kv(0)
        qts, _ = load_qt(0)
        prev_wo_last = None
        for b in range(B):
            # issue b+1's loads BEFORE anything in batch b can block the queues
            kv_next = load_kv(b + 1) if b + 1 < B else None
            qt_ld = None
            if b + 1 < B:
                qts_next, qt_ld = load_qt(b + 1)
            else:
                qts_next = None
            last_mm = None
            ktf, vf = kv
            for h in range(HLOC):
                last_mm = _attn_head(g, pools, b, h, ktf, vf, qts)
            kv, qts = kv_next, qts_next
            nc.gpsimd.collective_compute(
                "AllGather",
                mybir.AluOpType.bypass,
                replica_groups=[list(range(NCORES))],
                ins=[g["ag_in"][b].opt()],
                outs=[g["ag_out"][b].opt()],
            )
            # W_o for batch b-1: keep it AFTER batch b's attention on the PE
            # queue (the scheduler otherwise hoists it and stalls on the AG)
            if b >= 1:
                prev_wo_last = g["wo_batch"](b - 1, last_mm, qt_ld, prev_wo_last)
        g["wo_batch"](3, None, None, prev_wo_last)


def _make_wo_batch(g, wo_all, agp, osp, wps):
    nc = g["nc"]
    from concourse.tile import add_dep_helper
    f32, bf16 = g["f32"], g["bf16"]

    def wo_batch(b, after_mm, after_dma, prev_last):
        last = None
        with nc.named_scope("wo"):
            for pass_ in range(2):
                po = [wps.tile([128, 512], f32, name=f"po{b}_{pass_}_{m}",
                               tag=f"po{m}") for m in range(2)]
                # 8 batched agt loads (4 k-chunks each) instead of 32
                ags = []
                for kg in range(8):
                    agt = agp.tile([128, 4 * 256], bf16,
                                   name=f"agt{b}_{pass_}_{kg}", tag="agt")
                    ld = nc.sync.dma_start(
                        agt[:].rearrange("p (k q) -> p k q", k=4),
                        g["ag_out"][b].rearrange("(k p) q -> p k q", p=128)[
                            :, kg * 4:(kg + 1) * 4,
                            pass_ * 256:(pass_ + 1) * 256],
                    )
                    if kg == 0 and pass_ == 0 and after_dma is not None:
                        add_dep_helper(ld.ins, after_dma.ins, sync=False,
                                       reason="agt loads after next-batch qt")
                    ags.append(agt)
                for k in range(KC):
                    agt = ags[k // 4]
                    coff = (k % 4) * 256
                    for m in range(2):
                        mm = nc.tensor.matmul(
                            po[m][:], agt[:, coff + m * 128:coff + (m + 1) * 128],
                            wo_all[:, k * 512:(k + 1) * 512],
                            start=(k == 0), stop=(k == KC - 1))
                        if k == 0:
                            for dep in (after_mm, prev_last):
                                if dep is not None:
                                    add_dep_helper(mm.ins, dep.ins, sync=False,
                                                   reason="wo after attention")
                        last = mm
                for m in range(2):
                    ost = osp.tile([128, 512], f32,
                                   name=f"os{b}_{pass_}_{m}", tag="os")
                    nc.vector.tensor_copy(ost[:], po[m][:])
                    row = (b * 4 + pass_ * 2 + m) * 128
                    nc.sync.dma_start(g["outp_d"][row:row + 128, :], ost[:])
        return last

    return wo_batch


def _body(g):
    nc, tc_ = g["nc"], g["tc"]
    f32, f32r, bf16 = g["f32"], g["f32r"], g["bf16"]
    pp, dp = g["pp"], g["dp"]

    g["kt_new"] = pp.tile([128, TOK], f32r, name="kt_new")   # (hd, t)
    g["vt_new"] = pp.tile([128, TOK], f32r, name="vt_new")   # (hd, t)
    g["v_nat"] = pp.tile([128, TOK], f32r, name="v_nat")     # (t%128, [tc, hd])
    mask_t = pp.tile([128, 2 * 1024], f32r, name="mask_t")
    nc.gpsimd.dma_start(
        mask_t[:].rearrange("p (c q) -> p c q", c=2),
        g["mk_d"].rearrange("c p q -> p c q"),
    )
    g["mask_t"] = mask_t
    ones_f = pp.tile([128, 128], f32, name="ones_f")
    nc.vector.memset(ones_f[:], 1.0)
    ones_c = pp.tile([128, 128], f32r, name="ones_c")
    nc.vector.tensor_copy(ones_c[:], ones_f[:])
    g["ones_c"] = ones_c
    ident = pp.tile([128, 128], f32, name="ident")
    g["make_identity"](nc, ident)
    g["ident"] = ident

    g["qt_d"] = dp.tile([HLOC, 128, TOK], f32r, name="qt_d")
    g["ag_in"] = [dp.tile([512, 512], bf16, name=f"ag_in{b}") for b in range(B)]
    g["ag_out"] = [
        dp.tile([D, 512], bf16, name=f"ag_out{b}", addr_space="Shared")
        for b in range(B)
    ]

    _phase_proj(g)

    with (
        tc_.tile_pool(name="ka", bufs=2) as ka,
        tc_.tile_pool(name="va", bufs=2) as va,
        tc_.tile_pool(name="ep", bufs=6) as ep,
        tc_.tile_pool(name="tp2", bufs=2) as tp2,
        tc_.tile_pool(name="qp", bufs=3) as qp,
        tc_.tile_pool(name="rp", bufs=2) as rp,
        tc_.tile_pool(name="onp", bufs=2) as onp,
        tc_.tile_pool(name="wop", bufs=1) as wop,
        tc_.tile_pool(name="agp", bufs=6) as agp,
        tc_.tile_pool(name="osp", bufs=2) as osp,
    ):
        wo_all = wop.tile([128, KC * 512], bf16, name="wo_all")
        nc.gpsimd.dma_start(
            wo_all[:].rearrange("p (k m) -> p k m", k=KC),
            g["wo_d"].rearrange("(k p) m -> p k m", p=128),
        )
        with (
            tc_.tile_pool(name="scps", bufs=3, space="PSUM") as scps,
            tc_.tile_pool(name="ops", bufs=1, space="PSUM") as ops,
            tc_.tile_pool(name="dps", bufs=2, space="PSUM") as dps,
            tc_.tile_pool(name="wps", bufs=1, space="PSUM") as wps,
        ):
            g["wo_batch"] = _make_wo_batch(g, wo_all, agp, osp, wps)
            pools = (qp, ep, tp2, rp, onp, scps, ops, dps)
            _phase_attn(g, pools, ka, va)


def _build():
    import concourse.mybir as mybir
    import concourse.tile as tile
    from concourse import bacc
    from concourse.masks import make_identity

    f32 = mybir.dt.float32
    f32r = mybir.dt.float32r
    bf16 = mybir.dt.bfloat16

    nc = bacc.Bacc("TRN2", target_bir_lowering=False, debug=False,
                   num_devices=NCORES)

    g = {
        "nc": nc, "mybir": mybir, "make_identity": make_identity,
        "f32": f32, "f32r": f32r, "bf16": bf16,
        "xT_d": nc.dram_tensor("xT", [D, TOK], f32, kind="ExternalInput").ap(),
        "wq_d": nc.dram_tensor("wq", [D, HLOC * HD], f32, kind="ExternalInput").ap(),
        "wk_d": nc.dram_tensor("wk", [D, HD], f32, kind="ExternalInput").ap(),
        "wv_d": nc.dram_tensor("wv", [D, HD], f32, kind="ExternalInput").ap(),
        "ktc_d": nc.dram_tensor("ktc", [B, HD, KV], f32, kind="ExternalInput").ap(),
        "vc_d": nc.dram_tensor("vc", [B, KV, HD], f32, kind="ExternalInput").ap(),
        "wo_d": nc.dram_tensor("wo", [D, 512], f32, kind="ExternalInput").ap(),
        "mk_d": nc.dram_tensor("mk", [2, 128, 1024], f32, kind="ExternalInput").ap(),
        "outp_d": nc.dram_tensor("outp", [TOK, 512], f32, kind="ExternalOutput").ap(),
        "knew_d": nc.dram_tensor("knew", [TOK, HD], f32, kind="ExternalOutput").ap(),
        "vnew_d": nc.dram_tensor("vnew", [TOK, HD], f32, kind="ExternalOutput").ap(),
    }

    with tile.TileContext(nc) as tc_:
        g["tc"] = tc_
        with (
            tc_.tile_pool(name="persist", bufs=1) as pp,
            tc_.tile_pool(name="dram", bufs=1, space="DRAM") as dp,
        ):
            g["pp"], g["dp"] = pp, dp
            _body(g)

    nc.compile()
    return nc


def _get_nc():
    if "nc" not in _BUILT:
        _BUILT["nc"] = _build()
    return _BUILT["nc"]


def _host_prep(x, k_cache, v_cache, W_q, W_k, W_v, W_o):
    x = np.asarray(x, np.float32)
    k_cache = np.asarray(k_cache, np.float32)
    v_cache = np.asarray(v_cache, np.float32)
    W_q = np.asarray(W_q, np.float32)
    W_k = np.asarray(W_k, np.float32)
    W_v = np.asarray(W_v, np.float32)
    W_o = np.asarray(W_o, np.float32)

    xT = np.ascontiguousarray(x.reshape(TOK, D).T)
    ktc = np.ascontiguousarray(k_cache.transpose(0, 1, 3, 2))  # (B, KVH, HD, KV)

    # causal mask pair-tiles for the last 4 key chunks
    q_idx = np.arange(Q)
    mk = np.zeros((2, 128, 1024), np.float32)
    for p_ in range(2):
        for hf in range(2):
            c_local = p_ * 2 + hf
            j_glob = c_local * 128 + np.arange(128)
            mk[p_, :, hf * Q:(hf + 1) * Q] = (
                q_idx[None, :] >= j_glob[:, None]).astype(np.float32)

    in_maps = []
    for i in range(NCORES):
        in_maps.append({
            "xT": xT,
            "wq": np.ascontiguousarray(W_q[:, i * 512:(i + 1) * 512]),
            "wk": np.ascontiguousarray(W_k[:, i * HD:(i + 1) * HD]),
            "wv": np.ascontiguousarray(W_v[:, i * HD:(i + 1) * HD]),
            "ktc": np.ascontiguousarray(ktc[:, i]),
            "vc": np.ascontiguousarray(v_cache[:, i]),
            "wo": np.ascontiguousarray(W_o[:, i * 512:(i + 1) * 512]),
            "mk": mk,
        })
    return in_maps, k_cache, v_cache


def _assemble(results, k_cache, v_cache):
    out = np.empty((TOK, D), np.float32)
    k_new = np.empty((B, KVH, Q, HD), np.float32)
    v_new = np.empty((B, KVH, Q, HD), np.float32)
    for i, r in enumerate(results):
        out[:, i * 512:(i + 1) * 512] = r["outp"]
        k_new[:, i] = r["knew"].reshape(B, Q, HD)
        v_new[:, i] = r["vnew"].reshape(B, Q, HD)
    k_full = np.concatenate([k_cache, k_new], axis=2)
    v_full = np.concatenate([v_cache, v_new], axis=2)
    return out.reshape(B, Q, D), k_full, v_full


def run_spmd(in_maps, **kwargs):
    from concourse import bass_utils
    nc = _get_nc()
    return bass_utils.run_bass_kernel_spmd(
        nc, in_maps, core_ids=list(range(NCORES)), **kwargs)


def kernel(x, k_cache, v_cache, W_q, W_k, W_v, W_o):
    in_maps, kc, vc = _host_prep(x, k_cache, v_cache, W_q, W_k, W_v, W_o)
    res = run_spmd(in_maps)
    return _assemble(res.results, kc, vc)
